# revision 50
# baseline (speedup 1.0000x reference)
"""Trainium2 Bass kernel for nn_NonLinearReadoutBlock (equivariant readout MLP).

Math (see reference):
  x [N,512] = 128 scalars | 128 vectors x 3 (x[:,128+3i+c] = x_v[n,i,c])
  h = x @ W1 * inv1 (+b1 on scalars)  -> 16 scalars, 16 gates, 16 vectors
  scalars = silu(..); gates = silu(..); gated_v = gates * h_v
  out = [scalars @ W2_s * inv2 + b2  |  gated_v . W2_v * inv2]  -> [N,13]

Strategy: pure data-parallel over 8 cores (12500 rows each, padded to 12800).
x is transposed on the host to [512, rows] so DMA delivers feature-major
tiles straight into SBUF; all matmuls run in float32r (TF32 rate).

HW constraints honoured here:
  - engine APs must start at a 32-aligned partition
  - matmul PSUM dst must start at partition 0
  - PE tile_size must be square (32/64/128)
  - DVE reads at most one PSUM operand
  - repeating a (stationary tile, tile_position=32) matmul inside multi-mm
    accumulation groups >=3x crashes the device -> out stage is ONE K=80
    matmul over a packed moving tile instead
psum_h layout: gates 0:16, scalars 32:48, h_v (3o+c interleave) 64:112.
mv packed tile: scalars 0:16, zeros 16:32, gated_v 32:80.
"""

import math
from contextlib import ExitStack

import numpy as np

import concourse.bass as bass
import concourse.bacc as bacc
import concourse.tile as tile
from concourse import mybir
from concourse.bass import MemorySpace
from concourse.bass_utils import run_bass_kernel_spmd

F32 = mybir.dt.float32
F32R = mybir.dt.float32r

N_CORES = 8
ROWS_PER_CORE = 12800          # 25 tiles x 512 rows
TILE_ROWS = 512
N_TILES = ROWS_PER_CORE // TILE_ROWS
D_IN = 512
H = 112                        # psum_h: gates 0:16, scalars 32:48, h_v 64:112
MV = 80                        # packed moving tile for the out matmul
D_OUT = 13

_CACHE = {}


def _build_program(act_func=None, repeats=1, flat=False):
    nc = bacc.Bacc("TRN2", target_bir_lowering=False, debug=True)
    x_d = nc.declare_dram_parameter("x", [D_IN, ROWS_PER_CORE], F32R, isOutput=False)
    w_d = nc.declare_dram_parameter("w", [128, 4, H], F32R, isOutput=False)
    repl_d = nc.declare_dram_parameter("repl", [16, 48], F32R, isOutput=False)
    w2cat_d = nc.declare_dram_parameter("w2cat", [MV, D_OUT], F32R, isOutput=False)
    b1g_d = nc.declare_dram_parameter("b1g", [16, 1], F32, isOutput=False)
    b1s_d = nc.declare_dram_parameter("b1s", [32, 1], F32, isOutput=False)
    b2_d = nc.declare_dram_parameter("b2", [D_OUT, 1], F32, isOutput=False)
    out_d = nc.declare_dram_parameter("out", [D_OUT, ROWS_PER_CORE], F32, isOutput=True)

    ACT = mybir.ActivationFunctionType
    if act_func is None:
        act_func = ACT.Silu

    with tile.TileContext(nc) as tc, ExitStack() as ctx:
        consts = ctx.enter_context(tc.tile_pool(name="consts", bufs=1))
        xpool = ctx.enter_context(tc.tile_pool(name="x", bufs=3))
        g16pool = ctx.enter_context(tc.tile_pool(name="g16", bufs=3))
        hvpool = ctx.enter_context(tc.tile_pool(name="hv", bufs=3))
        mvpool = ctx.enter_context(tc.tile_pool(name="mv", bufs=3))
        opool = ctx.enter_context(tc.tile_pool(name="outT", bufs=2))
        ps_h = ctx.enter_context(tc.tile_pool(name="ps_h", bufs=2, space=MemorySpace.PSUM))
        ps_ga = ctx.enter_context(tc.tile_pool(name="ps_ga", bufs=2, space=MemorySpace.PSUM))
        ps_gb = ctx.enter_context(tc.tile_pool(name="ps_gb", bufs=2, space=MemorySpace.PSUM))
        ps_o = ctx.enter_context(tc.tile_pool(name="ps_o", bufs=2, space=MemorySpace.PSUM))

        w_sb = consts.tile([128, 4, H], F32R)
        nc.sync.dma_start(out=w_sb, in_=w_d[:])
        repl_sb = consts.tile([16, 48], F32R)
        nc.sync.dma_start(out=repl_sb, in_=repl_d[:])
        w2cat_sb = consts.tile([MV, D_OUT], F32R)
        nc.sync.dma_start(out=w2cat_sb, in_=w2cat_d[:])
        b1g_sb = consts.tile([16, 1], F32)
        nc.sync.dma_start(out=b1g_sb, in_=b1g_d[:])
        b1s_sb = consts.tile([32, 1], F32)
        nc.sync.dma_start(out=b1s_sb, in_=b1s_d[:])
        b2_sb = consts.tile([D_OUT, 1], F32)
        nc.sync.dma_start(out=b2_sb, in_=b2_d[:])

        # [128 partitions, kb, rows]: partition p of block kb holds feature kb*128+p
        x_view = x_d[:, :].rearrange("(kb p) r -> p kb r", kb=4)

        total = repeats * N_TILES
        hv_t = [None] * total
        g16_t = [None] * total
        mv_t = [None] * total

        def emit_head(t):
            tt = t % N_TILES
            xs = xpool.tile([128, 4, TILE_ROWS], F32R)
            nc.sync.dma_start(out=xs, in_=x_view[:, :, tt * TILE_ROWS:(tt + 1) * TILE_ROWS])
            ph = ps_h.tile([H, TILE_ROWS], F32)
            for kb in range(4):
                nc.tensor.matmul(
                    ph,
                    w_sb[:, kb, :],
                    xs[:, kb, :],
                    start=(kb == 0),
                    stop=(kb == 3),
                )
            g16 = g16pool.tile([16, TILE_ROWS], F32R)
            nc.scalar.activation(g16, ph[0:16], act_func, bias=b1g_sb)
            mv = mvpool.tile([MV, TILE_ROWS], F32R)
            # ph[48:64] is exactly 0 (zero weight cols) and b1s rows 16:32 are 0,
            # so this also writes act(0)=0 into the mv[16:32] pad hole
            nc.scalar.activation(mv[0:32], ph[32:64], act_func, bias=b1s_sb)
            hv = hvpool.tile([48, TILE_ROWS], F32)
            nc.vector.tensor_copy(hv, ph[64:H])
            g16_t[t] = g16
            hv_t[t] = hv
            mv_t[t] = mv

        def emit_gate(t):
            # replicate 16 gates -> 48 lanes (3o+c) via two matmuls, both at
            # PSUM dst base 0 (separate tiles)
            pga = ps_ga.tile([32, TILE_ROWS], F32)
            nc.tensor.matmul(pga, repl_sb[:, 0:32], g16_t[t], start=True, stop=True)
            pgb = ps_gb.tile([16, TILE_ROWS], F32)
            nc.tensor.matmul(pgb, repl_sb[:, 32:48], g16_t[t], start=True, stop=True)
            mv = mv_t[t]
            nc.vector.tensor_mul(mv[32:64], pga, hv_t[t][0:32])
            nc.vector.tensor_mul(mv[64:80], pgb, hv_t[t][32:48])
            hv_t[t] = None
            g16_t[t] = None

        def emit_out(t):
            po = ps_o.tile([D_OUT, TILE_ROWS], F32)
            nc.tensor.matmul(po, w2cat_sb, mv_t[t], start=True, stop=True)
            outT = opool.tile([D_OUT, TILE_ROWS], F32)
            nc.scalar.activation(outT, po, ACT.Identity, bias=b2_sb)
            tt = t % N_TILES
            nc.sync.dma_start(out=out_d[:, tt * TILE_ROWS:(tt + 1) * TILE_ROWS], in_=outT)
            mv_t[t] = None

        if flat:
            for t in range(total):
                emit_head(t)
                emit_gate(t)
                emit_out(t)
        else:
            for t in range(total):
                emit_head(t)
                if t >= 1:
                    emit_gate(t - 1)
                if t >= 2:
                    emit_out(t - 2)
            emit_gate(total - 1)
            emit_out(total - 2)
            emit_out(total - 1)

    nc.finalize()
    return nc


def _host_weights(W1_s, W1_v, b1_s, W2_s, W2_v, b2_s):
    inv1 = 1.0 / math.sqrt(128.0)
    inv2 = 1.0 / math.sqrt(16.0)
    i = np.arange(128)
    o = np.arange(16)

    w_ext = np.zeros((D_IN, H), np.float32)
    w_ext[0:128, 0:16] = W1_s[:, 16:32] * inv1          # gates
    w_ext[0:128, 32:48] = W1_s[:, 0:16] * inv1          # scalars
    for c in range(3):
        w_ext[np.ix_(128 + 3 * i + c, 64 + 3 * o + c)] = W1_v * inv1
    w_t = np.ascontiguousarray(w_ext.reshape(4, 128, H).transpose(1, 0, 2))

    repl = np.zeros((16, 48), np.float32)
    for c in range(3):
        repl[o, 3 * o + c] = 1.0

    w2cat = np.zeros((MV, D_OUT), np.float32)
    w2cat[0:16, 0:10] = W2_s * inv2
    for c in range(3):
        w2cat[32 + 3 * o + c, 10 + c] = W2_v[:, 0] * inv2

    b1g = b1_s[16:32].reshape(16, 1).astype(np.float32)
    b1sc = np.zeros((32, 1), np.float32)
    b1sc[0:16, 0] = b1_s[0:16]

    b2e = np.zeros((D_OUT, 1), np.float32)
    b2e[0:10, 0] = b2_s
    return w_t, repl, w2cat, b1g, b1sc, b2e


def _in_maps(x, W1_s, W1_v, b1_s, W2_s, W2_v, b2_s):
    N = x.shape[0]
    total = N_CORES * ROWS_PER_CORE
    x_pad = np.zeros((total, D_IN), np.float32)
    x_pad[:N] = x
    # [cores, rows, feat] -> [cores, feat, rows]
    x_t = np.ascontiguousarray(
        x_pad.reshape(N_CORES, ROWS_PER_CORE, D_IN).transpose(0, 2, 1)
    )
    w_t, repl, w2cat, b1g, b1sc, b2e = _host_weights(W1_s, W1_v, b1_s, W2_s, W2_v, b2_s)
    return [
        {"x": x_t[i], "w": w_t, "repl": repl, "w2cat": w2cat,
         "b1g": b1g, "b1s": b1sc, "b2": b2e}
        for i in range(N_CORES)
    ]


def _run(x, W1_s, W1_v, b1_s, W2_s, W2_v, b2_s):
    if "nc" not in _CACHE:
        _CACHE["nc"] = _build_program()
    nc = _CACHE["nc"]

    N = x.shape[0]
    in_maps = _in_maps(x, W1_s, W1_v, b1_s, W2_s, W2_v, b2_s)
    res = run_bass_kernel_spmd(nc, in_maps, list(range(N_CORES)), trace=False)
    out = np.concatenate([res.results[i]["out"].T for i in range(N_CORES)], axis=0)[:N]
    return np.ascontiguousarray(out.astype(np.float32))


def kernel(**inputs):
    return _run(**inputs)
